# revision 12
# baseline (speedup 1.0000x reference)
"""Trainium2 Bass kernel for LorentzSelfAttention (B=8, L=2048, D=128, 1 head).

Sharding: data-parallel over batch — core b handles batch element b.

Per-core algorithm (one NeuronCore, L=2048, D=128, 16 row-chunks of 128):
  Inputs arrive HOST-TRANSPOSED: qT/kT/vT [D, L] so no on-device input
  transposes are needed (the contraction dim must sit on partitions).
  Phase B (projections), grouped 4 chunks at a time:
      linT chunk c: matmul(lhsT=xT[:, c*128:(c+1)*128], rhs=W^T) -> natural
      [l, dout] PSUM. Row-wise Lorentz stats (sigmoid / sum-of-squares) are
      batched into [128, 48] stat tiles so ONE ACT Sqrt serves all 48 chunks
      (Sigmoid, Sqrt, Exp live in different ACT table sets — interleaving
      them costs a ~1.3us table load per switch; Square/Copy are in every
      set and are free). q/k chunks are PE-transposed into qT/kT [d, l];
      v stays natural with padded rows zeroed.
  Phase C (attention): scores computed TRANSPOSED, S_T[j, i] = <k_j, q_i>_L
      (q time row negated), only causal columns i >= j*128, float32r matmuls.
      exp() applied without max-subtract / row-sum normalization: the final
      Lorentz mid-point normalization out = ave/sqrt(|<ave,ave>_L|) is
      scale-invariant per row, so all softmax constants cancel. Pad masking
      is folded into v (zeroed rows); causal diag-block masking is a 0/1
      upper-tri multiply. AV accumulates transposed: outT[d, i] += v_j.T @
      expT_j (PSUM, 16 steps, float32r).
  Phase D: PE-transpose outT back to natural (grouped 4 chunks / PSUM bank),
      batched row-wise Lorentz normalize (one Sqrt), DMA out.

Rows whose allowed (causal & non-pad) key set is empty produce softmax over
an all -inf row in the reference (== uniform over ALL 2048 keys). Those rows
(a ~0-2 row prefix per batch, only when the batch's first keys are padded)
are fixed up exactly on host.
"""

import os

import numpy as np

B, L, D = 8, 2048, 128
P = 128
NCHUNK = L // P   # 16
G = 4             # chunks per group
NGROUP = NCHUNK // G  # 4

_RUNNER_CACHE: dict = {}


def _bcast3(bass, ap2, inner):
    """[P, n] AP -> [P, n, inner] broadcast view (step-0 innermost)."""
    return bass.AP(tensor=ap2.tensor, offset=ap2.offset,
                   ap=[ap2.ap[0], ap2.ap[1], [0, inner]])


# ---------------------------------------------------------------- device code
def _build_program(cfg, consts):
    from contextlib import ExitStack

    import concourse.bacc as bacc
    import concourse.bass as bass
    import concourse.mybir as mybir
    import concourse.tile as tile
    from concourse import masks

    f32 = mybir.dt.float32
    f32r = mybir.dt.float32r
    AF = mybir.ActivationFunctionType
    OP = mybir.AluOpType

    es = {"q": consts["es_q"], "k": consts["es_k"], "v": consts["es_v"]}
    c1 = consts["c1"]
    has_bias = consts["has_bias"]

    def mmc(ap, kind):
        if cfg[kind] == "f32r":
            return ap.bitcast(f32r)
        return ap

    qk_dt = f32r if cfg["mm_qk"] == "f32r" else f32
    av_dt = f32r if cfg["mm_av"] == "f32r" else f32

    nc = bacc.Bacc("TRN2", target_bir_lowering=False, debug=False)

    xT_d = {}
    for nm in ("q", "k", "v"):
        xT_d[nm] = nc.dram_tensor(nm, [D, L], f32, kind="ExternalInput").ap()
    pad_d = nc.dram_tensor("pad", [L], f32, kind="ExternalInput").ap()
    wt_d = {nm: nc.dram_tensor(f"w{nm}t", [D, D], f32, kind="ExternalInput").ap()
            for nm in ("q", "k", "v")}
    bias_d = {}
    if has_bias:
        for nm in ("q", "k", "v"):
            bias_d[nm] = nc.dram_tensor(f"b{nm}", [1, D], f32,
                                        kind="ExternalInput").ap()
    out_d = nc.dram_tensor("out", [L, D], f32, kind="ExternalOutput").ap()

    TENSORS = ("q", "k", "v")

    with tile.TileContext(nc) as tc, ExitStack() as octx:
        cpool = octx.enter_context(tc.tile_pool(name="consts", bufs=1))
        ident = cpool.tile([P, P], f32)
        masks.make_identity(nc, ident[:])
        ut01 = cpool.tile([P, P], f32)
        masks.make_upper_triangular(nc, ut01[:], val=1.0, diag=True)
        w_sb = {}
        for nm in TENSORS:
            w_sb[nm] = cpool.tile([P, D], f32, name=f"w_{nm}", tag=f"w_{nm}")
            nc.sync.dma_start(out=w_sb[nm][:], in_=wt_d[nm][:, :])
        pad_sb = cpool.tile([P, NCHUNK], f32)
        nc.sync.dma_start(out=pad_sb[:], in_=pad_d.rearrange("(c p) -> p c", p=P))
        bias_sb = {}
        if has_bias:
            for nm in TENSORS:
                bt = cpool.tile([P, D], f32)
                bd = bias_d[nm]
                nc.sync.dma_start(out=bt[:], in_=bass.AP(
                    tensor=bd.tensor, offset=bd.offset, ap=[[0, P], bd.ap[1]]))
                bias_sb[nm] = bt

        # persistent activations
        qT_sb = cpool.tile([P, L], qk_dt)      # [d, l], time row negated
        kT_sb = cpool.tile([P, L], qk_dt)
        v_sb = cpool.tile([P, NCHUNK, D], av_dt)  # [l%128, chunk, d], pad-zeroed
        qk_nat = cpool.tile([P, 2, NCHUNK, D], f32)  # q/k chunks, natural
        outT_sb = cpool.tile([P, L], f32)

        # batched per-row stats: col t*16+c is chunk c of tensor t
        time_all = cpool.tile([P, 3 * NCHUNK], f32)
        ss_all = cpool.tile([P, 3 * NCHUNK], f32)
        sqs_all = cpool.tile([P, 3 * NCHUNK], f32)

        # ---------------- Phase B: projections ----------------
        with ExitStack() as ctxB:
            xin = ctxB.enter_context(tc.tile_pool(name="xin", bufs=3))
            ps_l = ctxB.enter_context(tc.tile_pool(name="ps_l", bufs=3, space="PSUM"))
            ps_q = ctxB.enter_context(tc.tile_pool(name="ps_q", bufs=2, space="PSUM"))
            misc = ctxB.enter_context(tc.tile_pool(name="misc", bufs=3))
            stat = ctxB.enter_context(tc.tile_pool(name="stat", bufs=4))

            def dest4(nm, g):
                if nm == "v":
                    return v_sb[:, g * G:(g + 1) * G, :]
                ti = TENSORS.index(nm)
                return qk_nat[:, ti, g * G:(g + 1) * G, :]

            # pass 1: matmuls + sigmoid/square stats (ACT stays in the
            # sigmoid table set: Sigmoid+Square only), narrow -> SBUF dest
            for g in range(NGROUP):
                for nm in TENSORS:
                    ti = TENSORS.index(nm)
                    sb = ti * NCHUNK + g * G   # stats col base
                    xT4 = xin.tile([P, G * P], f32, tag=f"x{nm}")
                    nc.sync.dma_start(
                        out=xT4[:], in_=xT_d[nm][:, g * G * P:(g + 1) * G * P])
                    lin4 = ps_l.tile([P, G * D], f32, tag="lin")
                    for c in range(G):
                        nc.tensor.matmul(
                            lin4[:, c * D:(c + 1) * D],
                            mmc(xT4[:, c * P:(c + 1) * P], "mm_proj"),
                            mmc(w_sb[nm][:], "mm_proj"), start=True, stop=True)
                    if has_bias:
                        lin_sb4 = misc.tile([P, G * D], f32, tag="linb")
                        nc.vector.tensor_add(
                            lin_sb4[:], lin4[:],
                            bass.AP(tensor=bias_sb[nm].tensor,
                                    offset=bias_sb[nm][:].offset,
                                    ap=[bias_sb[nm][:].ap[0], [0, G], [1, D]]))
                        src4 = lin_sb4[:].rearrange("p (c d) -> p c d", d=D)
                    else:
                        src4 = lin4[:].rearrange("p (c d) -> p c d", d=D)
                    # sigmoid of column 0 -> sig (into time_all slot, raw)
                    nc.scalar.activation(
                        time_all[:, sb:sb + G], src4[:, :, 0:1], AF.Sigmoid)
                    # sum of squares of narrow part
                    sq4 = misc.tile([P, G, D - 1], f32, tag="sq4")
                    nc.scalar.activation(sq4[:], src4[:, :, 1:D], AF.Square)
                    nc.vector.tensor_reduce(
                        ss_all[:, sb:sb + G], sq4[:], mybir.AxisListType.X,
                        OP.add)
                    # park unscaled narrow in its SBUF destination (frees PSUM)
                    nc.vector.tensor_copy(dest4(nm, g)[:, :, 1:D],
                                          src4[:, :, 1:D])

            # pass 2: batched scalar math over all 48 chunks
            #   time = sig*exp(s) + 1.1  (per-tensor scale)
            for nm in TENSORS:
                ti = TENSORS.index(nm)
                sb = ti * NCHUNK
                nc.vector.tensor_scalar(
                    out=time_all[:, sb:sb + NCHUNK],
                    in0=time_all[:, sb:sb + NCHUNK],
                    scalar1=es[nm], scalar2=1.1, op0=OP.mult, op1=OP.add)
            inv_ss = stat.tile([P, 3 * NCHUNK], f32, tag="iss")
            nc.vector.reciprocal(inv_ss[:], ss_all[:])
            t2m1 = stat.tile([P, 3 * NCHUNK], f32, tag="t2m1")
            nc.vector.tensor_mul(t2m1[:], time_all[:], time_all[:])
            nc.vector.tensor_scalar_add(out=t2m1[:], in0=t2m1[:], scalar1=-1.0)
            sval = stat.tile([P, 3 * NCHUNK], f32, tag="sval")
            nc.vector.tensor_mul(sval[:], t2m1[:], inv_ss[:])
            nc.scalar.activation(sqs_all[:], sval[:], AF.Sqrt)
            # fold pad zeroing into v scales (time + narrow) — AFTER the sqrt
            vb = 2 * NCHUNK
            nc.vector.tensor_mul(
                sqs_all[:, vb:vb + NCHUNK], sqs_all[:, vb:vb + NCHUNK], pad_sb[:])
            nc.vector.tensor_mul(
                time_all[:, vb:vb + NCHUNK], time_all[:, vb:vb + NCHUNK],
                pad_sb[:])

            # pass 3: finish chunks in place, transpose q/k into qT/kT
            for g in range(NGROUP):
                for nm in TENSORS:
                    ti = TENSORS.index(nm)
                    sb = ti * NCHUNK + g * G
                    ch4 = dest4(nm, g)
                    tsign = -1.0 if nm == "q" else 1.0
                    # signed time into col 0
                    nc.vector.tensor_scalar(
                        out=ch4[:, :, 0:1], in0=time_all[:, sb:sb + G],
                        scalar1=tsign, scalar2=0.0, op0=OP.mult, op1=OP.add)
                    # narrow scaled by sqrt(s) in place (per-chunk broadcast)
                    nc.vector.tensor_mul(
                        ch4[:, :, 1:D], ch4[:, :, 1:D],
                        _bcast3(bass, sqs_all[:, sb:sb + G], D - 1))
                    if nm != "v":
                        qkT4 = ps_q.tile([P, G * P], f32, tag="qkT")
                        for c in range(G):
                            nc.tensor.transpose(
                                qkT4[:, c * P:(c + 1) * P], ch4[:, c, :],
                                ident[:])
                        dst = qT_sb if nm == "q" else kT_sb
                        nc.vector.tensor_copy(
                            dst[:, g * G * P:(g + 1) * G * P], qkT4[:])

        # ---------------- Phase C: attention ----------------
        with ExitStack() as ctxC:
            ps_s = ctxC.enter_context(tc.tile_pool(name="ps_s", bufs=2, space="PSUM"))
            ps_o = ctxC.enter_context(tc.tile_pool(name="ps_o", bufs=1, space="PSUM"))
            sb_e = ctxC.enter_context(tc.tile_pool(name="sb_e", bufs=2))
            outT_ps = ps_o.tile([P, L], f32)

            for j in range(NCHUNK):
                ncols = (NCHUNK - j) * P
                base = j * P
                expT = sb_e.tile([P, L], av_dt, tag="expT")  # col0 == global i=base
                kblk = kT_sb[:, base:base + P]
                ofs = 0
                while ofs < ncols:   # scores + exp in <=1024-col slabs
                    sw = min(1024, ncols - ofs)
                    s_ps = ps_s.tile([P, 1024], f32, tag="s")
                    mofs = 0
                    while mofs < sw:  # matmul N<=512 per PSUM bank
                        w = min(512, sw - mofs)
                        nc.tensor.matmul(
                            s_ps[:, mofs:mofs + w], kblk,
                            qT_sb[:, base + ofs + mofs:base + ofs + mofs + w],
                            start=True, stop=True)
                        mofs += w
                    nc.scalar.activation(
                        expT[:, ofs:ofs + sw], s_ps[:, :sw], AF.Exp, scale=c1)
                    ofs += sw
                # causal mask inside the diagonal block
                nc.vector.tensor_mul(expT[:, 0:P], expT[:, 0:P], ut01[:])
                # outT[d, i] += v_j.T @ expT_j  (bank-aligned psum chunks)
                col = base
                while col < L:
                    bank_end = min(L, (col // 512 + 1) * 512)
                    kbank = bank_end // 512 - 1
                    last_j = 4 * kbank + 3
                    nc.tensor.matmul(
                        outT_ps[:, col:bank_end],
                        v_sb[:, j, :],
                        expT[:, col - base:bank_end - base],
                        start=(j == 0), stop=(j == last_j))
                    col = bank_end

            nc.vector.tensor_copy(outT_sb[:], outT_ps[:])

        # ---------------- Phase D: normalize + store ----------------
        with ExitStack() as ctxD:
            ps_d = ctxD.enter_context(tc.tile_pool(name="ps_d", bufs=4, space="PSUM"))
            dmisc = ctxD.enter_context(tc.tile_pool(name="dmisc", bufs=3))
            dstat = ctxD.enter_context(tc.tile_pool(name="dstat", bufs=2))
            na_all = dstat.tile([P, NCHUNK], f32, tag="na")
            rn_all = dstat.tile([P, NCHUNK], f32, tag="rn")
            o_keep = {}
            for g in range(NGROUP):
                o_ps4 = ps_d.tile([P, G, D], f32, tag="o")
                o_keep[g] = o_ps4
                for c in range(G):
                    nc.tensor.transpose(
                        o_ps4[:, c, :],
                        outT_sb[:, (g * G + c) * P:(g * G + c + 1) * P],
                        ident[:])
                scr4 = dmisc.tile([P, G, D], f32, tag="scr")
                nc.scalar.activation(scr4[:], o_ps4[:], AF.Square)
                # ssum (t^2 + |n|^2) then na = -lor = 2*t^2 - ssum
                nc.vector.tensor_reduce(
                    na_all[:, g * G:(g + 1) * G], scr4[:],
                    mybir.AxisListType.X, OP.add)
                nc.vector.tensor_scalar(
                    out=scr4[:, :, 0:1], in0=scr4[:, :, 0:1], scalar1=2.0,
                    scalar2=0.0, op0=OP.mult, op1=OP.add)
                nc.vector.tensor_sub(
                    na_all[:, g * G:(g + 1) * G], scr4[:, :, 0:1],
                    na_all[:, g * G:(g + 1) * G])
            sq_na = dstat.tile([P, NCHUNK], f32, tag="sqna")
            nc.scalar.activation(sq_na[:], na_all[:], AF.Sqrt)
            nc.vector.reciprocal(rn_all[:], sq_na[:])
            for g in range(NGROUP):
                o_sb4 = dmisc.tile([P, G, D], f32, tag="osb")
                nc.vector.tensor_mul(
                    o_sb4[:], o_keep[g][:],
                    _bcast3(bass, rn_all[:, g * G:(g + 1) * G], D))
                nc.sync.dma_start(
                    out=out_d[g * G * P:(g + 1) * G * P, :].rearrange(
                        "(c p) d -> p c d", p=P),
                    in_=o_sb4[:])

    nc.compile()
    return nc


def _get_runner(cfg_key, consts):
    if cfg_key in _RUNNER_CACHE:
        return _RUNNER_CACHE[cfg_key]
    cfg = dict(mm_qk=consts["mm_qk"], mm_av=consts["mm_av"],
               mm_proj=consts["mm_proj"])
    nc = _build_program(cfg, consts)
    _RUNNER_CACHE[cfg_key] = nc
    return nc


# ---------------------------------------------------------------- host logic
def _host_fixup_rows(out, value, mask, Wv, bv, sv):
    """Exactly reproduce reference for rows with no allowed keys."""
    for b in range(B):
        cnt = np.cumsum(~mask[b])
        rows = np.where(cnt == 0)[0]
        if rows.size == 0:
            continue
        x = value[b].astype(np.float32) @ Wv.T.astype(np.float32) + bv
        time = 1.0 / (1.0 + np.exp(-x[:, :1])) * np.exp(sv) + 1.1
        xn = x[:, 1:]
        s = (time * time - 1.0) / np.sum(xn * xn, axis=-1, keepdims=True)
        vproj = np.concatenate([time, xn * np.sqrt(s)], axis=-1)
        ave = vproj.mean(axis=0)
        lor = -ave[0] ** 2 + np.sum(ave[1:] ** 2)
        denom = np.sqrt(max(abs(lor), 1e-8))
        out[b, rows] = (ave / denom).astype(np.float32)


def kernel(query, key, value, mask, Wq, bq, sq, Wk, bk, sk, Wv, bv, sv,
           attn_scale, attn_bias):
    from concourse.bass_utils import run_bass_kernel_spmd

    query = np.asarray(query, dtype=np.float32)
    key = np.asarray(key, dtype=np.float32)
    value = np.asarray(value, dtype=np.float32)
    mask = np.asarray(mask).astype(bool)
    Wq, Wk, Wv = (np.asarray(w, dtype=np.float32) for w in (Wq, Wk, Wv))
    bq, bk, bv = (np.asarray(b, dtype=np.float32).reshape(-1)
                  for b in (bq, bk, bv))

    has_bias = bool(np.any(bq) or np.any(bk) or np.any(bv))
    consts = dict(
        es_q=float(np.exp(np.float32(sq))),
        es_k=float(np.exp(np.float32(sk))),
        es_v=float(np.exp(np.float32(sv))),
        c1=float(2.0 / np.asarray(attn_scale, dtype=np.float32).reshape(-1)[0]),
        has_bias=has_bias,
        mm_qk=os.environ.get("LK_MM_QK", "f32r"),
        mm_av=os.environ.get("LK_MM_AV", "f32"),
        mm_proj=os.environ.get("LK_MM_PROJ", "f32"),
    )
    cfg_key = tuple(sorted(consts.items()))
    nc = _get_runner(cfg_key, consts)

    pad01 = (~mask).astype(np.float32)
    wt = {"q": np.ascontiguousarray(Wq.T), "k": np.ascontiguousarray(Wk.T),
          "v": np.ascontiguousarray(Wv.T)}
    in_maps = []
    for b in range(B):
        m = {
            "q": np.ascontiguousarray(query[b].T),
            "k": np.ascontiguousarray(key[b].T),
            "v": np.ascontiguousarray(value[b].T),
            "pad": pad01[b],
            "wqt": wt["q"], "wkt": wt["k"], "wvt": wt["v"],
        }
        if has_bias:
            m["bq"] = bq.reshape(1, D)
            m["bk"] = bk.reshape(1, D)
            m["bv"] = bv.reshape(1, D)
        in_maps.append(m)

    res = run_bass_kernel_spmd(nc, in_maps, core_ids=list(range(B)))
    out = np.stack([res.results[b]["out"] for b in range(B)], axis=0)
    _host_fixup_rows(out, value, mask, Wv, bv, float(np.float32(sv)))
    return out
